# revision 18
# baseline (speedup 1.0000x reference)
"""Trainium2 Bass kernel for the CustomRNN problem.

Math (per batch row):
    h_t   = tanh(x_t @ W1 + b1)                 (parallel over t)
    y_t   = h_t + tanh(y_{t-1} @ W2 + b2)       (serial scan over t)
    out_t = y_t @ Wc + bc                       (parallel over t)

Strategy (8 cores, data-parallel over batch; B_LOC = 32 rows/core):
  * On-chip activations live in "transposed" layout [U, n].  h uses
    b-major columns (n = b*T + t, the natural GEMM1 output order);
    tau uses t-major columns (n = t*32 + b) so the scan's ACT writes,
    z-matmul reads and classifier weight loads are all contiguous.
  * Scan recurrence rewritten so the serial critical path is exactly
    PE -> ACT -> PE per step (one bf16 matmul + one tanh):
        g_t   = h_t @ W2            (parallel GEMM, accumulated directly
                                     into the scan PSUM banks)
        tau_t = tanh(s_t + b2)      (ACT, PSUM -> SBUF)
        s_{t+1} = g_t + tau_t @ W2  (PE matmul accumulate, start=False)
  * y = h + tau is never materialized: the classifier computes
    out = h @ Wc + tau @ Wc as two accumulating matmuls per tile,
    interleaved into the scan's idle PE windows.
  * x is transposed on-chip with PE transpose-mode matmuls (the DMA
    xbar path serializes ~1.3us/tile globally).
  * All heavy matmuls are bf16 (fp32 matmuls lower to 2x hi/lo
    LDWEIGHTS+MATMUL passes on trn2); accumulation stays fp32 in PSUM.
"""

import contextlib

import numpy as np

import concourse.bacc as bacc
import concourse.bass as bass
import concourse.mybir as mybir
import concourse.tile as tile
from concourse import bass_utils
from concourse.masks import make_identity

B, T, D, U, C = 256, 512, 128, 128, 64
NCORES = 8
BL = B // NCORES  # 32 batch rows per core
P = 128
SLOTS = 16  # scan slots per PSUM bank
NBANKS = T // SLOTS  # 32

f32 = mybir.dt.float32
bf16 = mybir.dt.bfloat16
Tanh = mybir.ActivationFunctionType.Tanh


def build_body(nc, tc, ctx, x, w1d, b1d, w2d, b2d, wcd, bcd, outd, rep=0):
    pfx = f"r{rep}_"
    const = ctx.enter_context(tc.tile_pool(name=pfx + "const", bufs=1))
    big = ctx.enter_context(tc.tile_pool(name=pfx + "big", bufs=1))

    # ---- constants ----
    w1f = const.tile([D, U], f32)
    nc.sync.dma_start(w1f[:], w1d[:])
    w1s = const.tile([D, U], bf16)
    nc.vector.tensor_copy(w1s[:], w1f[:])
    w2f = const.tile([U, U], f32)
    nc.sync.dma_start(w2f[:], w2d[:])
    w2s = const.tile([U, U], bf16)
    nc.vector.tensor_copy(w2s[:], w2f[:])
    wcf = const.tile([U, C], f32)
    nc.sync.dma_start(wcf[:], wcd[:])
    wcb = const.tile([U, C], bf16)
    nc.vector.tensor_copy(wcb[:], wcf[:])
    b1s = const.tile([U, 1], f32)
    nc.sync.dma_start(b1s[:], b1d.unsqueeze(1))
    b2s = const.tile([U, 1], f32)
    nc.sync.dma_start(b2s[:], b2d.unsqueeze(1))
    zero32 = const.tile([U, BL], f32)
    nc.vector.memset(zero32[:], 0.0)
    ones1 = const.tile([1, P], f32)
    nc.vector.memset(ones1[:], 1.0)
    bc1 = const.tile([1, C], f32)
    nc.sync.dma_start(bc1[:], bcd.unsqueeze(0))
    idn = const.tile([P, P], bf16, name="idn")
    make_identity(nc, idn)

    # ---- big SBUF buffers ----
    hbuf = big.tile([P, BL * T], bf16)  # h, b-major columns
    taub = big.tile([P, BL * T], bf16)  # tau, t-major columns
    # strided view of h in (t, b) order for the g-matmul rhs
    Hv = hbuf[:].rearrange("p (b t) -> p t b", b=BL, t=T)
    # h columns for classifier tile k (t in [4k, 4k+4), all b), (t', b) order
    Hc = hbuf[:].rearrange("p (b tk t4) -> p tk t4 b", b=BL, t4=4)

    # output rows for classifier tile k: rows (t4, b) interleaved
    # outd is [BL, T, C]; row index = b*T + 4k + t'
    Ov = outd.rearrange("b (tk t4) c -> tk t4 b c", t4=4)

    # ---- phase A: x load, cast, PE-transpose, input GEMM ----
    xa_pool = ctx.enter_context(tc.tile_pool(name=pfx + "xa", bufs=3))
    xb_pool = ctx.enter_context(tc.tile_pool(name=pfx + "xb", bufs=3))
    xt_pool = ctx.enter_context(tc.tile_pool(name=pfx + "xt", bufs=3))

    with tc.tile_pool(name=pfx + "ph", bufs=2, space="PSUM") as ph_psum, \
         tc.tile_pool(name=pfx + "tp", bufs=2, space="PSUM") as tp_psum:
        # bc broadcast tile via K=1 matmul (bcb4 = ones^T @ bc, tiled 4x)
        psmall = ph_psum.tile([P, C], f32, tag="ph")
        nc.tensor.matmul(psmall[:], lhsT=ones1[:], rhs=bc1[:], start=True,
                         stop=True)
        bcb4 = const.tile([P, 4 * C], f32)
        for k in range(4):
            nc.vector.tensor_copy(bcb4[:, k * C:(k + 1) * C], psmall[:])

        for b in range(BL):
            xa = xa_pool.tile([P, T], f32)
            # x[b] is [T, D]; rows t = a*128 + p onto partition p
            nc.sync.dma_start(xa[:], x[b].rearrange("(a p) d -> p a d", p=P))
            xb = xb_pool.tile([P, T], bf16)
            nc.vector.tensor_copy(xb[:], xa[:])
            xt = xt_pool.tile([P, T], bf16)
            for a in range(4):
                # PE transpose: [128(t'),128(d)] -> psum [128(d),128(t')]
                tp = tp_psum.tile([P, P], bf16, tag="tp")
                nc.tensor.transpose(tp[:], xb[:, a * P:(a + 1) * P], idn[:])
                nc.vector.tensor_copy(xt[:, a * P:(a + 1) * P], tp[:])
            ph = ph_psum.tile([P, T], f32, tag="ph")
            nc.tensor.matmul(ph[:], lhsT=w1s[:], rhs=xt[:], start=True,
                             stop=True)
            nc.scalar.activation(hbuf[:, b * T:(b + 1) * T], ph[:], Tanh,
                                 bias=b1s[:])

    # ---- phase B: serial scan with classifier interleaved ----
    scan_psum = ctx.enter_context(
        tc.tile_pool(name=pfx + "scan", bufs=4, space="PSUM"))
    cls_psum = ctx.enter_context(
        tc.tile_pool(name=pfx + "cls", bufs=2, space="PSUM"))
    osb_pool = ctx.enter_context(tc.tile_pool(name=pfx + "osb", bufs=3))
    yst_pool = ctx.enter_context(tc.tile_pool(name=pfx + "yst", bufs=3))

    # tau_0 = tanh(0 + b2); tau_t = taub[:, t*BL:(t+1)*BL]
    nc.scalar.activation(taub[:, 0:BL], zero32[:], Tanh, bias=b2s[:])

    bank = None
    cps = None
    for t in range(T):
        m, sl = divmod(t, SLOTS)
        if sl == 0:
            bank = scan_psum.tile([P, SLOTS * BL], f32, tag="bank")
            # g for this bank: slot sl' holds g_{16m+sl'} = h_{16m+sl'} @ W2
            nc.tensor.matmul(
                bank[:],
                lhsT=w2s[:],
                rhs=Hv[:, m * SLOTS:(m + 1) * SLOTS, :],
                start=True,
                stop=False,
                skip_group_check=True,
            )
        slot = bank[:, sl * BL:(sl + 1) * BL]
        if t < T - 1:
            # s_{t+1} += tau_t @ W2
            nc.tensor.matmul(
                slot,
                lhsT=w2s[:],
                rhs=taub[:, t * BL:(t + 1) * BL],
                start=False,
                stop=True,
                skip_group_check=True,
            )
            # tau_{t+1} = tanh(s_{t+1} + b2)
            nc.scalar.activation(taub[:, (t + 1) * BL:(t + 2) * BL], slot,
                                 Tanh, bias=b2s[:])
        if t % 4 == 3:
            # classifier tile k: out rows (t', b) for t in [4k, 4k+4)
            # y = h + tau staged on DVE (t-major contiguous), then 1 matmul
            k = t // 4
            yst = yst_pool.tile([P, P], bf16)
            nc.vector.tensor_add(yst[:], taub[:, k * P:(k + 1) * P],
                                 Hc[:, k, :, :])
            if k % 4 == 0:
                cps = cls_psum.tile([P, 4 * C], f32, tag="cls")
            nc.tensor.matmul(
                cps[:, (k % 4) * C:(k % 4 + 1) * C],
                lhsT=yst[:],
                rhs=wcb[:],
                start=True,
                stop=True,
                skip_group_check=True,
            )
            if k % 4 == 3:
                osb = osb_pool.tile([P, 4 * C], f32)
                nc.vector.tensor_add(osb[:], cps[:], bcb4[:])
                for kk in range(k - 3, k + 1):
                    nc.sync.dma_start(
                        Ov[kk], osb[:, (kk % 4) * C:(kk % 4 + 1) * C])


def build_nc(nrep=1):
    nc = bacc.Bacc("TRN2", target_bir_lowering=False, debug=False,
                   num_devices=NCORES)
    x = nc.dram_tensor("inputs", [BL, T, D], f32, kind="ExternalInput").ap()
    w1 = nc.dram_tensor("W1", [D, U], f32, kind="ExternalInput").ap()
    b1 = nc.dram_tensor("b1", [U], f32, kind="ExternalInput").ap()
    w2 = nc.dram_tensor("W2", [U, U], f32, kind="ExternalInput").ap()
    b2 = nc.dram_tensor("b2", [U], f32, kind="ExternalInput").ap()
    wc = nc.dram_tensor("Wc", [U, C], f32, kind="ExternalInput").ap()
    bc = nc.dram_tensor("bc", [C], f32, kind="ExternalInput").ap()
    out = nc.dram_tensor("out", [BL, T, C], f32, kind="ExternalOutput").ap()

    with tile.TileContext(nc) as tc:
        for rep in range(nrep):
            with contextlib.ExitStack() as ctx:
                build_body(nc, tc, ctx, x, w1, b1, w2, b2, wc, bc, out,
                           rep=rep)
    nc.finalize()
    return nc


def make_in_maps(inputs):
    xs = np.ascontiguousarray(np.asarray(inputs["inputs"], dtype=np.float32))
    shards = np.split(xs, NCORES, axis=0)
    common = {
        k: np.ascontiguousarray(np.asarray(inputs[k], dtype=np.float32))
        for k in ("W1", "b1", "W2", "b2", "Wc", "bc")
    }
    return [dict(inputs=shards[i], **common) for i in range(NCORES)]


def kernel(**inputs):
    nc = build_nc()
    in_maps = make_in_maps(inputs)
    res = bass_utils.run_bass_kernel_spmd(nc, in_maps, list(range(NCORES)))
    outs = [np.asarray(res.results[i]["out"]) for i in range(NCORES)]
    return np.concatenate(outs, axis=0).astype(np.float32)


# revision 19
# speedup vs baseline: 1.0388x; 1.0388x over previous
"""Trainium2 Bass kernel for the CustomRNN problem.

Math (per batch row):
    h_t   = tanh(x_t @ W1 + b1)                 (parallel over t)
    y_t   = h_t + tanh(y_{t-1} @ W2 + b2)       (serial scan over t)
    out_t = y_t @ Wc + bc                       (parallel over t)

Strategy (8 cores, data-parallel over batch; B_LOC = 32 rows/core):
  * On-chip activations live in "transposed" layout [U, n].  h uses
    b-major columns (n = b*T + t, the natural GEMM1 output order);
    tau uses t-major columns (n = t*32 + b) so the scan's ACT writes,
    z-matmul reads and classifier weight loads are all contiguous.
  * Scan recurrence rewritten so the serial critical path is exactly
    PE -> ACT -> PE per step (one bf16 matmul + one tanh):
        g_t   = h_t @ W2            (parallel GEMM, accumulated directly
                                     into the scan PSUM banks)
        tau_t = tanh(s_t + b2)      (ACT, PSUM -> SBUF)
        s_{t+1} = g_t + tau_t @ W2  (PE matmul accumulate, start=False)
  * y = h + tau is never materialized: the classifier computes
    out = h @ Wc + tau @ Wc as two accumulating matmuls per tile,
    interleaved into the scan's idle PE windows.
  * x is transposed on-chip with PE transpose-mode matmuls (the DMA
    xbar path serializes ~1.3us/tile globally).
  * All heavy matmuls are bf16 (fp32 matmuls lower to 2x hi/lo
    LDWEIGHTS+MATMUL passes on trn2); accumulation stays fp32 in PSUM.
"""

import contextlib

import numpy as np

import concourse.bacc as bacc
import concourse.bass as bass
import concourse.mybir as mybir
import concourse.tile as tile
from concourse import bass_utils
from concourse.masks import make_identity

B, T, D, U, C = 256, 512, 128, 128, 64
NCORES = 8
BL = B // NCORES  # 32 batch rows per core
P = 128
SLOTS = 16  # scan slots per PSUM bank
NBANKS = T // SLOTS  # 32

f32 = mybir.dt.float32
bf16 = mybir.dt.bfloat16
Tanh = mybir.ActivationFunctionType.Tanh


def build_body(nc, tc, ctx, x, w1d, b1d, w2d, b2d, wcd, bcd, outd, rep=0):
    pfx = f"r{rep}_"
    const = ctx.enter_context(tc.tile_pool(name=pfx + "const", bufs=1))
    big = ctx.enter_context(tc.tile_pool(name=pfx + "big", bufs=1))

    # ---- constants ----
    w1f = const.tile([D, U], f32)
    nc.sync.dma_start(w1f[:], w1d[:])
    w1s = const.tile([D, U], bf16)
    nc.vector.tensor_copy(w1s[:], w1f[:])
    w2f = const.tile([U, U], f32)
    nc.sync.dma_start(w2f[:], w2d[:])
    w2s = const.tile([U, U], bf16)
    nc.vector.tensor_copy(w2s[:], w2f[:])
    wcf = const.tile([U, C], f32)
    nc.sync.dma_start(wcf[:], wcd[:])
    wcb = const.tile([U, C], bf16)
    nc.vector.tensor_copy(wcb[:], wcf[:])
    b1s = const.tile([U, 1], f32)
    nc.sync.dma_start(b1s[:], b1d.unsqueeze(1))
    b2s = const.tile([U, 1], f32)
    nc.sync.dma_start(b2s[:], b2d.unsqueeze(1))
    zero32 = const.tile([U, BL], f32)
    nc.vector.memset(zero32[:], 0.0)
    ones1 = const.tile([1, P], f32)
    nc.vector.memset(ones1[:], 1.0)
    bc1 = const.tile([1, C], f32)
    nc.sync.dma_start(bc1[:], bcd.unsqueeze(0))
    idn = const.tile([P, P], bf16, name="idn")
    make_identity(nc, idn)

    # ---- big SBUF buffers ----
    hbuf = big.tile([P, BL * T], bf16)  # h, b-major columns
    taub = big.tile([P, BL * T], bf16)  # tau, t-major columns
    # strided view of h in (t, b) order for the g-matmul rhs
    Hv = hbuf[:].rearrange("p (b t) -> p t b", b=BL, t=T)
    # h columns for classifier tile k (t in [4k, 4k+4), all b), (t', b) order
    Hc = hbuf[:].rearrange("p (b tk t4) -> p tk t4 b", b=BL, t4=4)

    # output rows for classifier tile k: rows (t4, b) interleaved
    # outd is [BL, T, C]; row index = b*T + 4k + t'
    Ov = outd.rearrange("b (tk t4) c -> tk t4 b c", t4=4)

    # ---- phase A: x load, cast, PE-transpose, input GEMM ----
    xa_pool = ctx.enter_context(tc.tile_pool(name=pfx + "xa", bufs=3))
    xb_pool = ctx.enter_context(tc.tile_pool(name=pfx + "xb", bufs=3))
    xt_pool = ctx.enter_context(tc.tile_pool(name=pfx + "xt", bufs=3))

    with tc.tile_pool(name=pfx + "ph", bufs=2, space="PSUM") as ph_psum, \
         tc.tile_pool(name=pfx + "tp", bufs=2, space="PSUM") as tp_psum:
        # bc broadcast tile via K=1 matmul (bcb4 = ones^T @ bc, tiled 4x)
        psmall = ph_psum.tile([P, C], f32, tag="ph")
        nc.tensor.matmul(psmall[:], lhsT=ones1[:], rhs=bc1[:], start=True,
                         stop=True)
        bcb4 = const.tile([P, 4 * C], f32)
        for k in range(4):
            nc.vector.tensor_copy(bcb4[:, k * C:(k + 1) * C], psmall[:])

        for b in range(BL):
            xa = xa_pool.tile([P, T], f32)
            # x[b] is [T, D]; rows t = a*128 + p onto partition p
            nc.sync.dma_start(xa[:], x[b].rearrange("(a p) d -> p a d", p=P))
            xb = xb_pool.tile([P, T], bf16)
            nc.vector.tensor_copy(xb[:], xa[:])
            xt = xt_pool.tile([P, T], bf16)
            for a in range(4):
                # PE transpose: [128(t'),128(d)] -> psum [128(d),128(t')]
                tp = tp_psum.tile([P, P], bf16, tag="tp")
                nc.tensor.transpose(tp[:], xb[:, a * P:(a + 1) * P], idn[:])
                nc.vector.tensor_copy(xt[:, a * P:(a + 1) * P], tp[:])
            ph = ph_psum.tile([P, T], f32, tag="ph")
            nc.tensor.matmul(ph[:], lhsT=w1s[:], rhs=xt[:], start=True,
                             stop=True)
            nc.scalar.activation(hbuf[:, b * T:(b + 1) * T], ph[:], Tanh,
                                 bias=b1s[:])

    # ---- phase B: serial scan with classifier interleaved ----
    scan_psum = ctx.enter_context(
        tc.tile_pool(name=pfx + "scan", bufs=4, space="PSUM"))
    cls_psum = ctx.enter_context(
        tc.tile_pool(name=pfx + "cls", bufs=3, space="PSUM"))
    osb_pool = ctx.enter_context(tc.tile_pool(name=pfx + "osb", bufs=10))
    yst_pool = ctx.enter_context(tc.tile_pool(name=pfx + "yst", bufs=10))

    # tau_0 = tanh(0 + b2); tau_t = taub[:, t*BL:(t+1)*BL]
    nc.scalar.activation(taub[:, 0:BL], zero32[:], Tanh, bias=b2s[:])

    bank = None
    cps = None
    for t in range(T):
        m, sl = divmod(t, SLOTS)
        if sl == 0:
            bank = scan_psum.tile([P, SLOTS * BL], f32, tag="bank")
            # g for this bank: slot sl' holds g_{16m+sl'} = h_{16m+sl'} @ W2
            nc.tensor.matmul(
                bank[:],
                lhsT=w2s[:],
                rhs=Hv[:, m * SLOTS:(m + 1) * SLOTS, :],
                start=True,
                stop=False,
                skip_group_check=True,
            )
        slot = bank[:, sl * BL:(sl + 1) * BL]
        if t < T - 1:
            # s_{t+1} += tau_t @ W2
            nc.tensor.matmul(
                slot,
                lhsT=w2s[:],
                rhs=taub[:, t * BL:(t + 1) * BL],
                start=False,
                stop=True,
                skip_group_check=True,
            )
            # tau_{t+1} = tanh(s_{t+1} + b2)
            nc.scalar.activation(taub[:, (t + 1) * BL:(t + 2) * BL], slot,
                                 Tanh, bias=b2s[:])
        if t % 4 == 3:
            # classifier tile k: out rows (t', b) for t in [4k, 4k+4)
            # y = h + tau staged on DVE (t-major contiguous), then 1 matmul
            k = t // 4
            yst = yst_pool.tile([P, P], bf16)
            nc.vector.tensor_add(yst[:], taub[:, k * P:(k + 1) * P],
                                 Hc[:, k, :, :])
            if k % 4 == 0:
                cps = cls_psum.tile([P, 4 * C], f32, tag="cls")
            nc.tensor.matmul(
                cps[:, (k % 4) * C:(k % 4 + 1) * C],
                lhsT=yst[:],
                rhs=wcb[:],
                start=True,
                stop=True,
                skip_group_check=True,
            )
            if k % 4 == 3:
                osb = osb_pool.tile([P, 4 * C], f32)
                nc.vector.tensor_add(osb[:], cps[:], bcb4[:])
                for kk in range(k - 3, k + 1):
                    nc.sync.dma_start(
                        Ov[kk], osb[:, (kk % 4) * C:(kk % 4 + 1) * C])


def build_nc(nrep=1):
    nc = bacc.Bacc("TRN2", target_bir_lowering=False, debug=False,
                   num_devices=NCORES)
    x = nc.dram_tensor("inputs", [BL, T, D], f32, kind="ExternalInput").ap()
    w1 = nc.dram_tensor("W1", [D, U], f32, kind="ExternalInput").ap()
    b1 = nc.dram_tensor("b1", [U], f32, kind="ExternalInput").ap()
    w2 = nc.dram_tensor("W2", [U, U], f32, kind="ExternalInput").ap()
    b2 = nc.dram_tensor("b2", [U], f32, kind="ExternalInput").ap()
    wc = nc.dram_tensor("Wc", [U, C], f32, kind="ExternalInput").ap()
    bc = nc.dram_tensor("bc", [C], f32, kind="ExternalInput").ap()
    out = nc.dram_tensor("out", [BL, T, C], f32, kind="ExternalOutput").ap()

    with tile.TileContext(nc) as tc:
        for rep in range(nrep):
            with contextlib.ExitStack() as ctx:
                build_body(nc, tc, ctx, x, w1, b1, w2, b2, wc, bc, out,
                           rep=rep)
    nc.finalize()
    return nc


def make_in_maps(inputs):
    xs = np.ascontiguousarray(np.asarray(inputs["inputs"], dtype=np.float32))
    shards = np.split(xs, NCORES, axis=0)
    common = {
        k: np.ascontiguousarray(np.asarray(inputs[k], dtype=np.float32))
        for k in ("W1", "b1", "W2", "b2", "Wc", "bc")
    }
    return [dict(inputs=shards[i], **common) for i in range(NCORES)]


def kernel(**inputs):
    nc = build_nc()
    in_maps = make_in_maps(inputs)
    res = bass_utils.run_bass_kernel_spmd(nc, in_maps, list(range(NCORES)))
    outs = [np.asarray(res.results[i]["out"]) for i in range(NCORES)]
    return np.concatenate(outs, axis=0).astype(np.float32)
